# revision 1
# baseline (speedup 1.0000x reference)
"""Distributed exact k-NN (FAISS IndexFlatL2 semantics) on 8 Trainium2 cores.

Strategy (per the standard distributed exact-kNN recipe):
 - Host: transpose the memory bank to [D, N] layout, shard along N across the
   8 cores, and precompute centered half-squared-norms so the device ranks by
   score = q.m - 0.5*(||m||^2 - D)  (a per-query-constant shift of -d2/2).
 - Device (SPMD, one shard per core): float32r (fast fp32) matmuls compute
   score tiles into PSUM (bias folded in via a K=1 matmul), ScalarE evicts
   tiles to SBUF, and the DVE max/max_index ops extract the top-8 candidates
   (value + index) per 2500-wide slab per query.  One output DMA returns all
   candidates.
 - Host: gathers the per-core candidates, keeps the best 16 per core per
   query, rescores them exactly in float64, and reduces to the global top-k
   (ties broken by lower index, matching jax.lax.top_k).

The per-slab top-8 cut is exact up to score noise: a true global top-5 item
is always within the top 5 of its own slab, and the float32r score noise
(~0.03 in d2 units) cannot push it below rank 8 of a 2500-item slab except
with negligible probability; the 16-per-core host cut has even more margin.
"""

import numpy as np

B, N, D = 256, 500000, 512
NCORES = 8
NLOC = N // NCORES          # 62500 rows per core
FT = 500                    # matmul tile width (one PSUM bank, >=256 for fp32r full rate)
SLAB = 2500                 # DVE max/max_index scan width
NCHUNK = D // 128           # 4 contraction chunks
TOPC = 16                   # candidates kept per core per query on the host

_built = None


def _split_multi_waits(nc):
    """This toolchain's walrus accepts at most one sem-wait/update per
    instruction; Tile attaches full lists.  Split extras into adjacent NoOps
    on the same engine (sequencers execute in order, so this is equivalent)."""
    import concourse.mybir as mybir
    import bass_rust

    counter = [0]
    dma_ops = {"DMACopy", "DMATranspose", "TensorLoad", "TensorSave", "DMAGather"}

    def nop(engine, wait=None, update=None):
        counter[0] += 1
        n = mybir.InstNoOp(name=f"WSPL-{counter[0]}")
        n.engine = engine
        n.sync_info = bass_rust.SyncInfo(
            on_wait=[wait] if wait is not None else [],
            on_update=[update] if update is not None else [],
        )
        return n

    for f in nc.m.functions:
        for bb in f.blocks:
            out = []
            changed = False
            for ins in bb.instructions:
                si = ins.sync_info
                if si is None:
                    out.append(ins)
                    continue
                waits = list(si.on_wait or [])
                updates = list(si.on_update or [])
                if len(waits) <= 1 and len(updates) <= 1:
                    out.append(ins)
                    continue
                changed = True
                for w in waits[:-1]:
                    out.append(nop(ins.engine, wait=w))
                keep_wait = waits[-1:] if waits else []
                if len(updates) > 1:
                    assert ins.opcode not in dma_ops, (
                        f"cannot split updates on DMA inst {ins.name}"
                    )
                    ins.sync_info = bass_rust.SyncInfo(
                        on_wait=keep_wait, on_update=updates[:1]
                    )
                    out.append(ins)
                    for u in updates[1:]:
                        out.append(nop(ins.engine, update=u))
                else:
                    ins.sync_info = bass_rust.SyncInfo(
                        on_wait=keep_wait, on_update=updates
                    )
                    out.append(ins)
            if changed:
                bb.instructions = out


def _build():
    """Build and cache the Bass program (identical for all cores)."""
    global _built
    if _built is not None:
        return _built
    import concourse.bass as bass
    import concourse.tile as tile
    import concourse.mybir as mybir

    nt = NLOC // FT             # matmul tiles per core
    nslab = NLOC // SLAB        # DVE slabs per core
    sub_per_slab = SLAB // FT
    cand = nslab * 8            # candidates per (core, query)
    f32r = mybir.dt.float32r
    f32 = mybir.dt.float32
    u32 = mybir.dt.uint32
    bf16 = mybir.dt.bfloat16

    nc = bass.Bass("TRN2", target_bir_lowering=False, debug=False)
    qT = nc.dram_tensor("qT", [D, B], bf16, kind="ExternalInput")
    memT = nc.dram_tensor("memT", [D, NLOC], bf16, kind="ExternalInput")
    msq = nc.dram_tensor("msq", [nslab, SLAB], f32, kind="ExternalInput")
    out = nc.dram_tensor("out", [128, 4 * cand], f32, kind="ExternalOutput")

    with tile.TileContext(nc) as tc:
        with tc.tile_pool(name="fixed", bufs=1) as fixed_pool, \
             tc.tile_pool(name="mem", bufs=3) as mem_pool, \
             tc.tile_pool(name="msq", bufs=3) as msq_pool, \
             tc.tile_pool(name="msqb", bufs=3) as msqb_pool, \
             tc.tile_pool(name="dist", bufs=3) as dist_pool, \
             tc.tile_pool(name="psum", bufs=6, space="PSUM") as psum_pool:

            qt = fixed_pool.tile([128, NCHUNK, B], bf16)
            nc.sync.dma_start(qt[:], qT.ap().rearrange("(c p) b -> p c b", p=128))
            outsb = fixed_pool.tile([128, 4 * cand], f32)

            memv = memT.ap().rearrange("(c p) n -> p c n", p=128)

            for slab in range(nslab):
                dist = [
                    dist_pool.tile([128, SLAB], f32, tag=f"dist{g}",
                                   name=f"dist{g}_{slab}")
                    for g in (0, 1)
                ]
                mem_t = mem_pool.tile([128, NCHUNK, SLAB], bf16)
                nc.sync.dma_start(
                    mem_t[:], memv[:, :, slab * SLAB:(slab + 1) * SLAB])
                msq_t = msq_pool.tile([1, SLAB], f32)
                nc.gpsimd.dma_start(msq_t[:], msq.ap()[slab:slab + 1, :])
                msqb = msqb_pool.tile([128, SLAB], f32, tag="msqb",
                                      name=f"msqb_{slab}")
                nc.gpsimd.dma_start(msqb[0:1, :], msq_t[:])
                for i in range(7):
                    w = 1 << i
                    nc.gpsimd.dma_start(msqb[w:2 * w, :], msqb[0:w, :])
                for g in (0, 1):
                    pss = [psum_pool.tile([128, FT], f32, tag="ps",
                                          name=f"ps_{slab}_{g}_{s_}")
                           for s_ in range(sub_per_slab)]
                    for c in range(NCHUNK):
                        for sub in range(sub_per_slab):
                            nc.tensor.matmul(
                                pss[sub][:],
                                qt[:, c, g * 128:(g + 1) * 128],
                                mem_t[:, c, sub * FT:(sub + 1) * FT],
                                start=(c == 0), stop=(c == NCHUNK - 1),
                            )
                    for sub in range(sub_per_slab):
                        nc.scalar.copy(dist[g][:, sub * FT:(sub + 1) * FT],
                                       pss[sub][:])
                    nc.vector.tensor_add(dist[g][:], dist[g][:], msqb[:])
                for g in (0, 1):
                    vs = outsb[:, g * cand + slab * 8: g * cand + slab * 8 + 8]
                    nc.vector.max(out=vs, in_=dist[g][:])
                    iv = outsb[:, (2 + g) * cand + slab * 8:
                               (2 + g) * cand + slab * 8 + 8].bitcast(u32)
                    nc.vector.max_index(iv, vs, dist[g][:])

            nc.sync.dma_start(out.ap(), outsb[:])

    _split_multi_waits(nc)
    _built = nc
    return nc


def _run_device(qT_np, memT_np, msqc_np, trace=False):
    """Run the SPMD program on all cores; returns (list of out arrays, exec_ns)."""
    from concourse.bass_utils import run_bass_kernel_spmd

    nc = _build()
    nt = NLOC // FT
    in_maps = []
    for c in range(NCORES):
        in_maps.append({
            "qT": qT_np,
            "memT": np.ascontiguousarray(memT_np[:, c * NLOC:(c + 1) * NLOC]),
            "msq": np.ascontiguousarray(
                msqc_np[c * NLOC:(c + 1) * NLOC].reshape(NLOC // SLAB, SLAB)),
        })
    res = run_bass_kernel_spmd(nc, in_maps, core_ids=list(range(NCORES)),
                               trace=trace)
    outs = [r["out"] for r in res.results]
    return outs, res.exec_time_ns


def kernel(query, memory, k, _trace=False, _return_exec=False):
    k = int(k)
    assert k <= 8
    import ml_dtypes
    query = np.asarray(query, dtype=np.float32)
    memory = np.asarray(memory, dtype=np.float32)
    nslab = NLOC // SLAB
    cand = nslab * 8

    # ---- host-side prep: transpose + centered half squared norms ----
    qT_np = np.ascontiguousarray(query.T).astype(ml_dtypes.bfloat16)   # [D, B]
    memT_np = np.ascontiguousarray(memory.T).astype(ml_dtypes.bfloat16)  # [D, N]
    msq = np.einsum("nd,nd->n", memory, memory)                # [N] fp32
    msqc_np = (-0.5 * (msq - float(D))).astype(np.float32)    # centered bias

    # ---- device: per-core approximate top-8 per slab ----
    outs, exec_ns = _run_device(qT_np, memT_np, msqc_np, trace=_trace)

    # ---- host: decode candidates, exact rescore, global top-k ----
    # per core: vals [B, cand], global idx [B, cand]
    all_vals = np.empty((NCORES, B, cand), dtype=np.float32)
    all_idx = np.empty((NCORES, B, cand), dtype=np.int64)
    slab_base = (np.arange(nslab).repeat(8) * SLAB).astype(np.int64)  # [cand]
    for c in range(NCORES):
        o = outs[c]
        for g in (0, 1):
            vals = o[:, g * cand:(g + 1) * cand]
            lidx = o[:, (2 + g) * cand:(3 + g) * cand].view(np.uint32)
            rows = slice(g * 128, (g + 1) * 128)
            all_vals[c, rows] = vals
            all_idx[c, rows] = c * NLOC + slab_base[None, :] + lidx

    # keep best TOPC per core per query (by approximate score, descending)
    keep = min(TOPC, cand)
    part = np.argpartition(-all_vals, keep - 1, axis=2)[:, :, :keep]
    cvals_idx = np.take_along_axis(all_idx, part, axis=2)      # [NCORES, B, keep]
    cand_idx = np.swapaxes(cvals_idx, 0, 1).reshape(B, NCORES * keep)

    # exact rescore in float64
    q64 = query.astype(np.float64)                             # [B, D]
    qsq = np.sum(q64 * q64, axis=1)                            # [B]
    flat = cand_idx.reshape(-1)
    mrows = memory[flat].astype(np.float64).reshape(B, NCORES * keep, D)
    cross = np.einsum("bd,bcd->bc", q64, mrows)
    msq64 = np.sum(mrows * mrows, axis=2)
    d2 = qsq[:, None] + msq64 - 2.0 * cross                    # [B, NCORES*keep]

    # dedupe is unnecessary (shards are disjoint, slabs are disjoint)
    distances = np.empty((B, k), dtype=np.float32)
    idx = np.empty((B, k), dtype=np.int32)
    for b in range(B):
        order = np.lexsort((cand_idx[b], d2[b]))[:k]
        distances[b] = d2[b][order].astype(np.float32)
        idx[b] = cand_idx[b][order].astype(np.int32)

    if _return_exec:
        return (distances, idx), exec_ns
    return distances, idx



# revision 2
# speedup vs baseline: 2.3362x; 2.3362x over previous
"""Distributed exact k-NN (FAISS IndexFlatL2 semantics) on 8 Trainium2 cores.

Strategy (v2 — window-pooled candidate generation):
 - Host: sort the memory bank by ||m||^2 (padding 500000 -> 524288 rows with
   zero rows that get bias -1e30), shard contiguous 65536-row blocks across
   the 8 cores, and transpose to [D, N] bf16.  Because rows are norm-sorted,
   every window of 16 consecutive rows has a nearly constant centered bias
   -0.5*(||m||^2 - D), so the bias can be applied AFTER max-pooling, to a
   16x smaller array.  A tiny per-window epsilon (w * 2^-14) makes pooled
   scores unique so FIND_INDEX8 never collapses duplicate needles.
 - Device (SPMD, one shard per core): bf16 matmuls compute q.m score tiles
   into PSUM (query groups g0/g1 own PSUM banks 0-3/4-7 so the PE never
   stalls on PSUM), ScalarE evicts tiles to SBUF bf16, the DVE max-pools
   windows of 16 (2x bf16 mode), and per 8-slab region adds the window bias
   and extracts the top-8 windows (value + index) with MAX8/FIND_INDEX8.
 - Host: gathers 8 windows x 4 regions x 8 cores = 256 candidate windows per
   query, keeps the best 28 by pooled score, exactly rescores their
   28*16=448 member rows in float64, and reduces to the global top-k
   (ties broken by lower index, matching jax.lax.top_k).

Correctness of the cut: a true global top-k item (k<=8) has core-rank <= 8,
so its window is within the top-8 pooled windows of its region (pigeonhole:
disjoint windows, each pooled value is a real item score).  The pooled-score
margins between a global top-5 item and a region's 8th-best window are tens
of d2 units, while bf16 score noise is <1 unit, so the cut survives rounding
with enormous margin.  The host keep-28-of-256 cut has the same property at
the global level (>=28 better windows would imply >=28 better items).
"""

import numpy as np

B, N, D = 256, 500000, 512
NCORES = 8
NPAD = 524288               # padded bank size (pads have bias -1e30)
NLOC = NPAD // NCORES       # 65536 rows per core
FT = 512                    # matmul tile width = one full PSUM bank
SUBS = 4                    # matmul tiles per slab
SLAB = FT * SUBS            # 2048: rows processed per (slab, group)
NSLAB = NLOC // SLAB        # 32 slabs per core
NCHUNK = D // 128           # 4 contraction chunks
W = 16                      # pooling window (consecutive norm-sorted rows)
NWIN = NLOC // W            # 4096 windows per core
SLABS_PER_REG = 8           # slabs per MAX8 scan region
NREG = NSLAB // SLABS_PER_REG   # 4 regions per core
REGW = NWIN // NREG         # 1024 windows per region
TOPW = 28                   # candidate windows kept per query on the host

_built = None


def _split_multi_waits(nc):
    """This toolchain's walrus accepts at most one sem-wait/update per
    instruction; Tile attaches full lists.  Split extras into adjacent NoOps
    on the same engine (sequencers execute in order, so this is equivalent)."""
    import concourse.mybir as mybir
    import bass_rust

    counter = [0]
    dma_ops = {"DMACopy", "DMATranspose", "TensorLoad", "TensorSave", "DMAGather"}

    def nop(engine, wait=None, update=None):
        counter[0] += 1
        n = mybir.InstNoOp(name=f"WSPL-{counter[0]}")
        n.engine = engine
        n.sync_info = bass_rust.SyncInfo(
            on_wait=[wait] if wait is not None else [],
            on_update=[update] if update is not None else [],
        )
        return n

    for f in nc.m.functions:
        for bb in f.blocks:
            out = []
            changed = False
            for ins in bb.instructions:
                si = ins.sync_info
                if si is None:
                    out.append(ins)
                    continue
                waits = list(si.on_wait or [])
                updates = list(si.on_update or [])
                if len(waits) <= 1 and len(updates) <= 1:
                    out.append(ins)
                    continue
                changed = True
                for w in waits[:-1]:
                    out.append(nop(ins.engine, wait=w))
                keep_wait = waits[-1:] if waits else []
                if len(updates) > 1:
                    assert ins.opcode not in dma_ops, (
                        f"cannot split updates on DMA inst {ins.name}"
                    )
                    ins.sync_info = bass_rust.SyncInfo(
                        on_wait=keep_wait, on_update=updates[:1]
                    )
                    out.append(ins)
                    for u in updates[1:]:
                        out.append(nop(ins.engine, update=u))
                else:
                    ins.sync_info = bass_rust.SyncInfo(
                        on_wait=keep_wait, on_update=updates
                    )
                    out.append(ins)
            if changed:
                bb.instructions = out


def _build():
    """Build and cache the Bass program (identical for all cores)."""
    global _built
    if _built is not None:
        return _built
    import concourse.bass as bass
    import concourse.tile as tile
    import concourse.mybir as mybir

    f32 = mybir.dt.float32
    u32 = mybir.dt.uint32
    bf16 = mybir.dt.bfloat16

    nc = bass.Bass("TRN2", target_bir_lowering=False, debug=False)
    qT = nc.dram_tensor("qT", [D, B], bf16, kind="ExternalInput")
    memT = nc.dram_tensor("memT", [D, NLOC], bf16, kind="ExternalInput")
    biasb = nc.dram_tensor("biasb", [128, NREG, REGW], f32, kind="ExternalInput")
    out = nc.dram_tensor("out", [128, 2 * NREG * 16], f32, kind="ExternalOutput")

    with tile.TileContext(nc) as tc:
        with tc.tile_pool(name="fixed", bufs=1) as fixed_pool, \
             tc.tile_pool(name="mem", bufs=4) as mem_pool, \
             tc.tile_pool(name="dist", bufs=2) as dist_pool, \
             tc.tile_pool(name="breg", bufs=2) as breg_pool, \
             tc.tile_pool(name="psum", bufs=1, space="PSUM") as psum_pool:

            qt = fixed_pool.tile([128, NCHUNK, B], bf16)
            nc.sync.dma_start(qt[:], qT.ap().rearrange("(c p) b -> p c b", p=128))
            biasb_t = fixed_pool.tile([128, NREG, REGW], f32)
            nc.sync.dma_start(biasb_t[:], biasb.ap())
            outsb = fixed_pool.tile([128, 2 * NREG * 16], f32)
            pooled = [
                fixed_pool.tile([128, NSLAB, SLAB // W], bf16, name=f"pooled{g}")
                for g in (0, 1)
            ]

            memv = memT.ap().rearrange("(c p) n -> p c n", p=128)

            for s in range(NSLAB):
                mem_t = mem_pool.tile([128, NCHUNK, SLAB], bf16, tag="mem",
                                      name=f"mem_{s}")
                nc.sync.dma_start(
                    mem_t[:], memv[:, :, s * SLAB:(s + 1) * SLAB])
                for g in (0, 1):
                    ps = psum_pool.tile([128, SUBS, 512], f32, tag=f"ps{g}",
                                        name=f"ps_{s}_{g}")
                    for c in range(NCHUNK):
                        for sub in range(SUBS):
                            nc.tensor.matmul(
                                ps[:, sub:sub + 1, 0:FT],
                                qt[:, c, g * 128:(g + 1) * 128],
                                mem_t[:, c, sub * FT:(sub + 1) * FT],
                                start=(c == 0), stop=(c == NCHUNK - 1),
                            )
                    dist = dist_pool.tile([128, SUBS, FT], bf16, tag=f"dist{g}",
                                          name=f"dist_{s}_{g}")
                    nc.scalar.copy(dist[:], ps[:, :, 0:FT])
                    nc.vector.tensor_reduce(
                        pooled[g][:, s:s + 1, :],
                        dist[:].rearrange("p a (w e) -> p a w e", e=W),
                        axis=mybir.AxisListType.X,
                        op=mybir.AluOpType.max,
                    )
                if s % SLABS_PER_REG == SLABS_PER_REG - 1:
                    r = s // SLABS_PER_REG
                    for g in (0, 1):
                        breg = breg_pool.tile([128, REGW], f32, tag="breg",
                                              name=f"breg_{r}_{g}")
                        nc.vector.tensor_add(
                            breg[:],
                            pooled[g][:, r * SLABS_PER_REG:(r + 1) * SLABS_PER_REG, :],
                            biasb_t[:, r:r + 1, :],
                        )
                        base = (g * NREG + r) * 16
                        vs = outsb[:, base:base + 8]
                        nc.vector.max(out=vs, in_=breg[:])
                        iv = outsb[:, base + 8:base + 16].bitcast(u32)
                        nc.vector.max_index(iv, vs, breg[:])

            nc.sync.dma_start(out.ap(), outsb[:])

    _split_multi_waits(nc)
    _built = nc
    return nc


def _run_device(qT_np, memT_list, biasb_list, trace=False):
    """Run the SPMD program on all cores; returns (list of out arrays, exec_ns)."""
    from concourse.bass_utils import run_bass_kernel_spmd

    nc = _build()
    in_maps = []
    for c in range(NCORES):
        in_maps.append({
            "qT": qT_np,
            "memT": memT_list[c],
            "biasb": biasb_list[c],
        })
    res = run_bass_kernel_spmd(nc, in_maps, core_ids=list(range(NCORES)),
                               trace=trace)
    outs = [r["out"] for r in res.results]
    return outs, res.exec_time_ns


def _prep_host(query, memory):
    """Norm-sort + pad the bank; build per-core bf16 shards and window biases.

    Returns (qT_np, memT_list, biasb_list, perm) where perm maps sorted-padded
    positions to original row ids (pads have id >= N).
    """
    import ml_dtypes

    qT_np = np.ascontiguousarray(query.T).astype(ml_dtypes.bfloat16)  # [D, B]

    msq = np.einsum("nd,nd->n", memory, memory)                # [N] fp32
    msq_pad = np.concatenate([msq, np.zeros(NPAD - N, np.float32)])
    # pads (msq 0) sort first; stable keeps their order
    perm = np.argsort(msq_pad, kind="stable")

    memT = np.ascontiguousarray(memory.T).astype(ml_dtypes.bfloat16)  # [D, N]

    memT_list, biasb_list = [], []
    eps = (np.arange(NWIN, dtype=np.float64) * 2.0 ** -14).astype(np.float32)
    for c in range(NCORES):
        cols = perm[c * NLOC:(c + 1) * NLOC]
        real = cols < N
        safe = np.where(real, cols, 0)
        mt = memT[:, safe]
        if not real.all():
            mt[:, ~real] = 0
        memT_list.append(np.ascontiguousarray(mt))

        win_msq = msq_pad[cols].reshape(NWIN, W).min(axis=1)
        bias = (-0.5 * (win_msq - float(D))).astype(np.float32) + eps
        pad_win = (~real).reshape(NWIN, W).any(axis=1)
        bias[pad_win] = -1e30
        bb = np.ascontiguousarray(
            np.broadcast_to(bias, (128, NWIN))).reshape(128, NREG, REGW)
        biasb_list.append(bb)
    return qT_np, memT_list, biasb_list, perm


def kernel(query, memory, k, _trace=False, _return_exec=False):
    k = int(k)
    assert k <= 8
    query = np.asarray(query, dtype=np.float32)
    memory = np.asarray(memory, dtype=np.float32)

    qT_np, memT_list, biasb_list, perm = _prep_host(query, memory)

    outs, exec_ns = _run_device(qT_np, memT_list, biasb_list, trace=_trace)

    # ---- host: decode candidate windows, exact rescore, global top-k ----
    # outs[c]: [128, 2*NREG*16] f32; layout [p, g, r, (8 vals | 8 idx)]
    o = np.stack(outs).reshape(NCORES, 128, 2, NREG, 16)
    vals = o[..., 0:8]                                    # [C, 128, 2, R, 8]
    widx = o[..., 8:16].view(np.uint32).astype(np.int64)  # region-local window
    # query b = g*128 + p  ->  arrange [B, C, R, 8]
    vals = np.transpose(vals, (2, 1, 0, 3, 4)).reshape(B, -1)
    widx = np.transpose(widx, (2, 1, 0, 3, 4)).reshape(B, -1)
    # global window id (within padded sorted space)
    core_of = np.repeat(np.arange(NCORES), NREG * 8)[None, :]   # [1, C*R*8]
    reg_of = np.tile(np.repeat(np.arange(NREG), 8), NCORES)[None, :]
    gwin = core_of * NWIN + reg_of * REGW + widx                # [B, C*R*8]

    # keep best TOPW windows per query by pooled score
    keep = np.argpartition(-vals, TOPW - 1, axis=1)[:, :TOPW]
    kwin = np.take_along_axis(gwin, keep, axis=1)               # [B, TOPW]

    # expand windows to member rows (original ids; pads -> excluded)
    rows = perm[(kwin[:, :, None] * W + np.arange(W)[None, None, :])
                .reshape(B, -1)]                                # [B, TOPW*W]
    valid = rows < N
    safe_rows = np.where(valid, rows, 0)

    # exact rescore in float64 (chunked over queries)
    q64 = query.astype(np.float64)
    qsq = np.sum(q64 * q64, axis=1)
    ncand = TOPW * W
    d2 = np.empty((B, ncand), dtype=np.float64)
    for b0 in range(0, B, 64):
        b1 = min(b0 + 64, B)
        mrows = memory[safe_rows[b0:b1].reshape(-1)].astype(np.float64)
        mrows = mrows.reshape(b1 - b0, ncand, D)
        cross = np.einsum("bd,bcd->bc", q64[b0:b1], mrows)
        msq64 = np.sum(mrows * mrows, axis=2)
        d2[b0:b1] = qsq[b0:b1, None] + msq64 - 2.0 * cross
    d2[~valid] = np.inf

    distances = np.empty((B, k), dtype=np.float32)
    idx = np.empty((B, k), dtype=np.int32)
    for b in range(B):
        order = np.lexsort((safe_rows[b], d2[b]))[:k]
        distances[b] = d2[b][order].astype(np.float32)
        idx[b] = safe_rows[b][order].astype(np.int32)

    if _return_exec:
        return (distances, idx), exec_ns
    return distances, idx


# revision 8
# speedup vs baseline: 3.6755x; 1.5733x over previous
"""Distributed exact k-NN (FAISS IndexFlatL2 semantics) on 8 Trainium2 cores.

Strategy (v2 — window-pooled candidate generation):
 - Host: sort the memory bank by ||m||^2 (padding 500000 -> 524288 rows with
   zero rows that get bias -1e30), shard contiguous 65536-row blocks across
   the 8 cores, and transpose to [D, N] bf16.  Because rows are norm-sorted,
   every window of 16 consecutive rows has a nearly constant centered bias
   -0.5*(||m||^2 - D), so the bias can be applied AFTER max-pooling, to a
   16x smaller array.  A tiny per-window epsilon (w * 2^-14) makes pooled
   scores unique so FIND_INDEX8 never collapses duplicate needles.
 - Device (SPMD, one shard per core): bf16 matmuls compute q.m score tiles
   into PSUM (query groups g0/g1 own PSUM banks 0-3/4-7 so the PE never
   stalls on PSUM), ScalarE evicts tiles to SBUF bf16, the DVE max-pools
   windows of 16 (2x bf16 mode), and per 8-slab region adds the window bias
   and extracts the top-8 windows (value + index) with MAX8/FIND_INDEX8.
 - Host: gathers 8 windows x 4 regions x 8 cores = 256 candidate windows per
   query, keeps the best 28 by pooled score, exactly rescores their
   28*16=448 member rows in float64, and reduces to the global top-k
   (ties broken by lower index, matching jax.lax.top_k).

Correctness of the cut: a true global top-k item (k<=8) has core-rank <= 8,
so its window is within the top-8 pooled windows of its region (pigeonhole:
disjoint windows, each pooled value is a real item score).  The pooled-score
margins between a global top-5 item and a region's 8th-best window are tens
of d2 units, while bf16 score noise is <1 unit, so the cut survives rounding
with enormous margin.  The host keep-28-of-256 cut has the same property at
the global level (>=28 better windows would imply >=28 better items).
"""

import numpy as np

B, N, D = 256, 500000, 512
NCORES = 8
NPAD = 524288               # padded bank size (pads have bias -1e30)
NLOC = NPAD // NCORES       # 65536 rows per core
FT = 512                    # matmul tile width = one full PSUM bank
SUBS = 4                    # matmul tiles per slab
SLAB = FT * SUBS            # 2048: rows processed per (slab, group)
NSLAB = NLOC // SLAB        # 32 slabs per core
NCHUNK = D // 128           # 4 contraction chunks
USE_FP8 = True              # fp8e4 DoubleRow matmuls (K=256/pass) vs bf16
W = 16                      # pooling window (consecutive norm-sorted rows)
NWIN = NLOC // W            # 4096 windows per core
SLABS_PER_REG = 8           # slabs per MAX8 scan region
NREG = NSLAB // SLABS_PER_REG   # 4 regions per core
REGW = NWIN // NREG         # 1024 windows per region
TOPW = 28                   # candidate windows kept per query on the host

_built = None


def _split_multi_waits(nc):
    """This toolchain's walrus accepts at most one sem-wait/update per
    instruction; Tile attaches full lists.  Split extras into adjacent NoOps
    on the same engine (sequencers execute in order, so this is equivalent)."""
    import concourse.mybir as mybir
    import bass_rust

    counter = [0]
    dma_ops = {"DMACopy", "DMATranspose", "TensorLoad", "TensorSave", "DMAGather"}

    def nop(engine, wait=None, update=None):
        counter[0] += 1
        n = mybir.InstNoOp(name=f"WSPL-{counter[0]}")
        n.engine = engine
        n.sync_info = bass_rust.SyncInfo(
            on_wait=[wait] if wait is not None else [],
            on_update=[update] if update is not None else [],
        )
        return n

    for f in nc.m.functions:
        for bb in f.blocks:
            out = []
            changed = False
            for ins in bb.instructions:
                si = ins.sync_info
                if si is None:
                    out.append(ins)
                    continue
                waits = list(si.on_wait or [])
                updates = list(si.on_update or [])
                if len(waits) <= 1 and len(updates) <= 1:
                    out.append(ins)
                    continue
                changed = True
                for w in waits[:-1]:
                    out.append(nop(ins.engine, wait=w))
                keep_wait = waits[-1:] if waits else []
                if len(updates) > 1:
                    assert ins.opcode not in dma_ops, (
                        f"cannot split updates on DMA inst {ins.name}"
                    )
                    ins.sync_info = bass_rust.SyncInfo(
                        on_wait=keep_wait, on_update=updates[:1]
                    )
                    out.append(ins)
                    for u in updates[1:]:
                        out.append(nop(ins.engine, update=u))
                else:
                    ins.sync_info = bass_rust.SyncInfo(
                        on_wait=keep_wait, on_update=updates
                    )
                    out.append(ins)
            if changed:
                bb.instructions = out


def _build():
    """Build and cache the Bass program (identical for all cores)."""
    global _built
    if _built is not None:
        return _built
    import concourse.bass as bass
    import concourse.tile as tile
    import concourse.mybir as mybir

    f32 = mybir.dt.float32
    u32 = mybir.dt.uint32
    bf16 = mybir.dt.bfloat16
    in_dt = mybir.dt.float8e4 if USE_FP8 else bf16

    nc = bass.Bass("TRN2", target_bir_lowering=False, debug=False)
    qT = nc.dram_tensor("qT", [D, B], in_dt, kind="ExternalInput")
    memT = nc.dram_tensor("memT", [D, NLOC], in_dt, kind="ExternalInput")
    biasb = nc.dram_tensor("biasb", [128, NREG, REGW], f32, kind="ExternalInput")
    out = nc.dram_tensor("out", [128, 2 * NREG * 16], f32, kind="ExternalOutput")

    with tile.TileContext(nc) as tc:
        with tc.tile_pool(name="fixed", bufs=1) as fixed_pool, \
             tc.tile_pool(name="mem", bufs=4) as mem_pool, \
             tc.tile_pool(name="dist", bufs=2) as dist_pool, \
             tc.tile_pool(name="breg", bufs=2) as breg_pool, \
             tc.tile_pool(name="fold", bufs=2) as fold_pool, \
             tc.tile_pool(name="psum", bufs=1, space="PSUM") as psum_pool:

            qt = fixed_pool.tile([128, NCHUNK, B], in_dt)
            nc.sync.dma_start(qt[:], qT.ap().rearrange("(c p) b -> p c b", p=128))
            biasb_t = fixed_pool.tile([128, NREG, REGW], f32)
            nc.sync.dma_start(biasb_t[:], biasb.ap())
            outsb = fixed_pool.tile([128, 2 * NREG * 16], f32)
            pooled = [
                fixed_pool.tile([128, NSLAB, SLAB // W], bf16, name=f"pooled{g}")
                for g in (0, 1)
            ]

            memv = memT.ap().rearrange("(c p) n -> p c n", p=128)

            for s in range(NSLAB):
                mem_t = mem_pool.tile([128, NCHUNK, SLAB], in_dt, tag="mem",
                                      name=f"mem_{s}")
                nc.sync.dma_start(
                    mem_t[:], memv[:, :, s * SLAB:(s + 1) * SLAB])
                for g in (0, 1):
                    ps = psum_pool.tile([128, SUBS, 512], f32, tag=f"ps{g}",
                                        name=f"ps_{s}_{g}")
                    if USE_FP8:
                        # DoubleRow: K=256 per pass via [128, 2, ...] operands
                        for c2 in range(NCHUNK // 2):
                            for sub in range(SUBS):
                                nc.tensor.matmul(
                                    ps[:, sub:sub + 1, 0:FT],
                                    qt[:, 2 * c2:2 * c2 + 2,
                                       g * 128:(g + 1) * 128],
                                    mem_t[:, 2 * c2:2 * c2 + 2,
                                          sub * FT:(sub + 1) * FT],
                                    start=(c2 == 0),
                                    stop=(c2 == NCHUNK // 2 - 1),
                                    perf_mode=mybir.MatmulPerfMode.DoubleRow,
                                )
                    else:
                        for c in range(NCHUNK):
                            for sub in range(SUBS):
                                nc.tensor.matmul(
                                    ps[:, sub:sub + 1, 0:FT],
                                    qt[:, c, g * 128:(g + 1) * 128],
                                    mem_t[:, c, sub * FT:(sub + 1) * FT],
                                    start=(c == 0), stop=(c == NCHUNK - 1),
                                )
                    dist = dist_pool.tile([128, SUBS, 32, W], bf16,
                                          tag=f"dist{g}", name=f"dist_{s}_{g}")
                    nc.scalar.copy(dist[:], ps[:, :, 0:FT])
                    # max-pool windows of 16 via TT-max folds (bf16 2x mode;
                    # TensorReduce has no 2x uop)
                    f1 = fold_pool.tile([128, SUBS, 32, 8], bf16, tag=f"f1{g}",
                                        name=f"f1_{s}_{g}")
                    nc.vector.tensor_max(
                        f1[:], dist[:, :, :, 0:8], dist[:, :, :, 8:16])
                    f2 = fold_pool.tile([128, SUBS, 32, 4], bf16, tag=f"f2{g}",
                                        name=f"f2_{s}_{g}")
                    nc.vector.tensor_max(
                        f2[:], f1[:, :, :, 0:4], f1[:, :, :, 4:8])
                    f3 = fold_pool.tile([128, SUBS, 32, 2], bf16, tag=f"f3{g}",
                                        name=f"f3_{s}_{g}")
                    nc.vector.tensor_max(
                        f3[:], f2[:, :, :, 0:2], f2[:, :, :, 2:4])
                    nc.vector.tensor_max(
                        pooled[g][:, s:s + 1, :],
                        f3[:, :, :, 0:1], f3[:, :, :, 1:2])
                if s % SLABS_PER_REG == SLABS_PER_REG - 1:
                    r = s // SLABS_PER_REG
                    for g in (0, 1):
                        breg = breg_pool.tile([128, REGW], f32, tag="breg",
                                              name=f"breg_{r}_{g}")
                        nc.vector.tensor_add(
                            breg[:],
                            pooled[g][:, r * SLABS_PER_REG:(r + 1) * SLABS_PER_REG, :],
                            biasb_t[:, r:r + 1, :],
                        )
                        base = (g * NREG + r) * 16
                        vs = outsb[:, base:base + 8]
                        nc.vector.max(out=vs, in_=breg[:])
                        iv = outsb[:, base + 8:base + 16].bitcast(u32)
                        nc.vector.max_index(iv, vs, breg[:])

            nc.sync.dma_start(out.ap(), outsb[:])

    _split_multi_waits(nc)
    _built = nc
    return nc


def _run_device(qT_np, memT_list, biasb_list, trace=False):
    """Run the SPMD program on all cores; returns (list of out arrays, exec_ns)."""
    from concourse.bass_utils import run_bass_kernel_spmd

    nc = _build()
    in_maps = []
    for c in range(NCORES):
        in_maps.append({
            "qT": qT_np,
            "memT": memT_list[c],
            "biasb": biasb_list[c],
        })
    res = run_bass_kernel_spmd(nc, in_maps, core_ids=list(range(NCORES)),
                               trace=trace)
    outs = [r["out"] for r in res.results]
    return outs, res.exec_time_ns


def _prep_host(query, memory):
    """Norm-sort + pad the bank; build per-core bf16 shards and window biases.

    Returns (qT_np, memT_list, biasb_list, perm) where perm maps sorted-padded
    positions to original row ids (pads have id >= N).
    """
    import ml_dtypes

    in_np_dt = ml_dtypes.float8_e4m3 if USE_FP8 else ml_dtypes.bfloat16
    qT_np = np.ascontiguousarray(query.T).astype(in_np_dt)     # [D, B]

    msq = np.einsum("nd,nd->n", memory, memory)                # [N] fp32
    msq_pad = np.concatenate([msq, np.zeros(NPAD - N, np.float32)])
    # pads (msq 0) sort first; stable keeps their order
    perm = np.argsort(msq_pad, kind="stable")

    memT = np.ascontiguousarray(memory.T).astype(in_np_dt)     # [D, N]

    memT_list, biasb_list = [], []
    eps = (np.arange(NWIN, dtype=np.float64) * 2.0 ** -14).astype(np.float32)
    for c in range(NCORES):
        cols = perm[c * NLOC:(c + 1) * NLOC]
        real = cols < N
        safe = np.where(real, cols, 0)
        mt = memT[:, safe]
        if not real.all():
            mt[:, ~real] = 0
        memT_list.append(np.ascontiguousarray(mt))

        win_msq = msq_pad[cols].reshape(NWIN, W).min(axis=1)
        bias = (-0.5 * (win_msq - float(D))).astype(np.float32) + eps
        pad_win = (~real).reshape(NWIN, W).any(axis=1)
        bias[pad_win] = -1e30
        bb = np.ascontiguousarray(
            np.broadcast_to(bias, (128, NWIN))).reshape(128, NREG, REGW)
        biasb_list.append(bb)
    return qT_np, memT_list, biasb_list, perm


def kernel(query, memory, k, _trace=False, _return_exec=False):
    k = int(k)
    assert k <= 8
    query = np.asarray(query, dtype=np.float32)
    memory = np.asarray(memory, dtype=np.float32)

    qT_np, memT_list, biasb_list, perm = _prep_host(query, memory)

    outs, exec_ns = _run_device(qT_np, memT_list, biasb_list, trace=_trace)

    # ---- host: decode candidate windows, exact rescore, global top-k ----
    # outs[c]: [128, 2*NREG*16] f32; layout [p, g, r, (8 vals | 8 idx)]
    o = np.stack(outs).reshape(NCORES, 128, 2, NREG, 16)
    vals = o[..., 0:8]                                    # [C, 128, 2, R, 8]
    widx = o[..., 8:16].view(np.uint32).astype(np.int64)  # region-local window
    # query b = g*128 + p  ->  arrange [B, C, R, 8]
    vals = np.transpose(vals, (2, 1, 0, 3, 4)).reshape(B, -1)
    widx = np.transpose(widx, (2, 1, 0, 3, 4)).reshape(B, -1)
    # global window id (within padded sorted space)
    core_of = np.repeat(np.arange(NCORES), NREG * 8)[None, :]   # [1, C*R*8]
    reg_of = np.tile(np.repeat(np.arange(NREG), 8), NCORES)[None, :]
    gwin = core_of * NWIN + reg_of * REGW + widx                # [B, C*R*8]

    # keep best TOPW windows per query by pooled score
    keep = np.argpartition(-vals, TOPW - 1, axis=1)[:, :TOPW]
    kwin = np.take_along_axis(gwin, keep, axis=1)               # [B, TOPW]

    # expand windows to member rows (original ids; pads -> excluded)
    rows = perm[(kwin[:, :, None] * W + np.arange(W)[None, None, :])
                .reshape(B, -1)]                                # [B, TOPW*W]
    valid = rows < N
    safe_rows = np.where(valid, rows, 0)

    # exact rescore in float64 (chunked over queries)
    q64 = query.astype(np.float64)
    qsq = np.sum(q64 * q64, axis=1)
    ncand = TOPW * W
    d2 = np.empty((B, ncand), dtype=np.float64)
    for b0 in range(0, B, 64):
        b1 = min(b0 + 64, B)
        mrows = memory[safe_rows[b0:b1].reshape(-1)].astype(np.float64)
        mrows = mrows.reshape(b1 - b0, ncand, D)
        cross = np.einsum("bd,bcd->bc", q64[b0:b1], mrows)
        msq64 = np.sum(mrows * mrows, axis=2)
        d2[b0:b1] = qsq[b0:b1, None] + msq64 - 2.0 * cross
    d2[~valid] = np.inf

    distances = np.empty((B, k), dtype=np.float32)
    idx = np.empty((B, k), dtype=np.int32)
    for b in range(B):
        order = np.lexsort((safe_rows[b], d2[b]))[:k]
        distances[b] = d2[b][order].astype(np.float32)
        idx[b] = safe_rows[b][order].astype(np.int32)

    if _return_exec:
        return (distances, idx), exec_ns
    return distances, idx
